# revision 1
# baseline (speedup 1.0000x reference)
"""Gated TCN layer (fully conditioned) as a Bass/Tile kernel on 8 NeuronCores.

Reference computation (per sample b):
    kern = (c @ adapter_w + adapter_b).reshape(2*CH, CH, K)
    y    = dilated causal conv of x with per-sample kern (K=3, dil=4)
    y   += (c @ bias_w + bias_b)[:, None]
    z    = tanh(y[:CH]) * sigmoid(y[CH:])
    out  = resi_w @ z + resi_b + x
Returns (out, z).

Sharding: data-parallel over batch. 16 samples / 8 cores = 2 samples per
core; the adapter / 1x1 weights are replicated. Matmuls run as float32r
(full PE rate for N>=256, fp32 storage).
"""

import numpy as np

from concourse import bacc, mybir, tile
from concourse.bass_utils import run_bass_kernel_spmd

K = 3
DIL = 4
CH = 64
COND = 128
B, T = 16, 16384
NCORES = 8
BL = B // NCORES          # samples per core
PAD = (K - 1) * DIL       # causal left pad = 8
NT = 512                  # matmul free-dim (one PSUM bank of fp32)
UW = 1024                 # processing unit width (2 PSUM banks)
NJ = T // UW
F = K * CH * 2 * CH       # 24576 adapter columns
FI = 2 * CH * K           # 384 adapter columns per input-channel row

F32 = mybir.dt.float32
F32R = mybir.dt.float32r
BF16 = mybir.dt.bfloat16
AF = mybir.ActivationFunctionType

# Adapter weights in bf16 (halves their DMA) vs f32r (more precise).
ADAPTER_BF16 = True

# Set by test.py to capture a profile; harness path leaves these alone.
TRACE = False
LAST_RESULTS = None

_NC = None


def _build():
    nc = bacc.Bacc("TRN2", target_bir_lowering=False, debug=False)

    x_in = nc.dram_tensor("x_in", [BL, CH, T], F32R, kind="ExternalInput")
    cT_d = nc.dram_tensor("cT", [COND, BL], BF16 if ADAPTER_BF16 else F32R, kind="ExternalInput")
    aw_d = nc.dram_tensor("aw_r", [COND, F], BF16 if ADAPTER_BF16 else F32R, kind="ExternalInput")
    ab_d = nc.dram_tensor("ab_r", [2 * CH, FI], F32R, kind="ExternalInput")
    cTf_d = nc.dram_tensor("cTf", [COND, BL], F32R, kind="ExternalInput")
    bw_d = nc.dram_tensor("bw", [COND, 2 * CH], F32R, kind="ExternalInput")
    bb_d = nc.dram_tensor("bb", [1, 2 * CH], F32R, kind="ExternalInput")
    riw_d = nc.dram_tensor("riw", [2 * CH, CH], F32R, kind="ExternalInput")
    rb_d = nc.dram_tensor("rb", [CH, 1], F32, kind="ExternalInput")
    out_d = nc.dram_tensor("out_d", [BL, CH, T], F32, kind="ExternalOutput")
    z_d = nc.dram_tensor("z_d", [BL, CH, T], F32R, kind="ExternalOutput")  # f32r == f32 bits

    with tile.TileContext(nc) as tc:
        with (
            tc.tile_pool(name="const", bufs=1) as constp,
            tc.tile_pool(name="xpool", bufs=1) as xpool,
            tc.tile_pool(name="kern", bufs=1) as kernp,
        ):
            # x for both samples packed on 128 partitions, left-padded by PAD.
            xbuf = xpool.tile([2 * CH, PAD + T], F32R)
            nc.vector.memset(xbuf[:, 0:PAD].bitcast(F32), 0.0)
            for b in range(BL):
                nc.sync.dma_start(
                    xbuf[CH * b : CH * (b + 1), PAD : PAD + T], x_in[b]
                )

            cT_sb = constp.tile([COND, BL], BF16 if ADAPTER_BF16 else F32R)
            nc.sync.dma_start(cT_sb[:, :], cT_d[:, :])
            cTf_sb = constp.tile([COND, BL], F32R)
            nc.sync.dma_start(cTf_sb[:, :], cTf_d[:, :])
            bw_sb = constp.tile([COND, 2 * CH], F32R)
            nc.sync.dma_start(bw_sb[:, :], bw_d[:, :])
            bb_sb = constp.tile([1, 2 * CH], F32R)
            nc.sync.dma_start(bb_sb[:, :], bb_d[:, :])
            riw_sb = constp.tile([2 * CH, CH], F32R)
            nc.sync.dma_start(riw_sb[:, :], riw_d[:, :])
            rb_sb = constp.tile([CH, 1], F32)
            nc.sync.dma_start(rb_sb[:, :], rb_d[:, :])
            ab_sb = constp.tile([2 * CH, FI], F32R)
            nc.sync.dma_start(ab_sb[:, :], ab_d[:, :])
            ones_sb = constp.tile([1, BL], F32R)
            nc.vector.memset(ones_sb[:, :].bitcast(F32), 1.0)

            # Raw (pre-bias) per-sample dynamic kernels, gathered row by row;
            # kfin = kern_raw + adapter_b, laid out [i, k*128 + o] so that
            # kfin[:, k*128:(k+1)*128] is the (i, o) lhsT of tap k.
            kern_raw = kernp.tile([2 * CH, FI], F32R, name="kern_raw")
            kfin = kernp.tile([2 * CH, FI], F32R, name="kfin")
            bias_sb = constp.tile([2 * CH, BL], F32)

            # ---------------- phase A: adapter + conditioned bias ----------
            with (
                tc.tile_pool(name="awp", bufs=3) as awp,
                tc.tile_pool(name="apsum", bufs=4, space="PSUM") as apsum,
                tc.tile_pool(name="stg", bufs=4) as stgp,
                tc.tile_pool(name="bpsum", bufs=1, space="PSUM") as bpsum,
            ):
                pb = bpsum.tile([2 * CH, BL], F32)
                nc.tensor.matmul(
                    pb[:, :], bw_sb[:, :], cTf_sb[:, :], start=True, stop=False
                )
                nc.tensor.matmul(
                    pb[:, :], bb_sb[:, :], ones_sb[:, :], start=False, stop=True
                )
                nc.vector.tensor_copy(bias_sb[:, :], pb[:, :])

                # 16 groups of 4 input-channel rows (4*FI = 1536 cols each)
                for g in range(CH // 4):
                    if g % 4 == 0:
                        awt = awp.tile([COND, 16 * FI], BF16 if ADAPTER_BF16 else F32R, tag="aw")
                        nc.sync.dma_start(
                            awt[:, :], aw_d[:, g * 4 * FI : (g + 16 // 4) * 4 * FI]
                        )
                        aoff = g * 4 * FI
                    stg = stgp.tile([BL, 4 * FI], F32R, tag="stg")
                    for u in range(4):
                        j = 4 * g + u
                        ps = apsum.tile([BL, FI], F32, tag="ap")
                        nc.tensor.matmul(
                            ps[:, :],
                            cT_sb[:, :],
                            awt[:, j * FI - aoff : (j + 1) * FI - aoff],
                            start=True,
                            stop=True,
                        )
                        if u % 2 == 0:
                            nc.scalar.activation(
                                stg[:, u * FI : (u + 1) * FI], ps[:, :], AF.Copy
                            )
                        else:
                            nc.vector.tensor_copy(
                                stg[:, u * FI : (u + 1) * FI], ps[:, :]
                            )
                    for b in range(BL):
                        nc.sync.dma_start(
                            kern_raw[CH * b + 4 * g : CH * b + 4 * g + 4, :],
                            stg[b : b + 1, :],
                        )
                nc.vector.tensor_add(kfin[:, :], kern_raw[:, :], ab_sb[:, :])

            # ---------------- phase B: conv + gate + residual --------------
            with (
                tc.tile_pool(name="ypsum", bufs=2, space="PSUM") as ypsum,
                tc.tile_pool(name="opsum", bufs=2, space="PSUM") as opsum,
                tc.tile_pool(name="work", bufs=3) as workp,
            ):
                for j in range(NJ):
                    for b in range(BL):
                        xrow = xbuf[CH * b : CH * (b + 1), :]
                        py = ypsum.tile([2 * CH, UW], F32, tag="py")
                        for h in range(UW // NT):
                            for k in range(K):
                                c0 = j * UW + h * NT + DIL * k
                                nc.tensor.matmul(
                                    py[:, h * NT : (h + 1) * NT],
                                    kfin[
                                        CH * b : CH * (b + 1),
                                        k * 2 * CH : (k + 1) * 2 * CH,
                                    ],
                                    xrow[:, c0 : c0 + NT],
                                    start=(k == 0),
                                    stop=(k == K - 1),
                                )
                        ta = workp.tile([CH, UW], F32R, tag="ta")
                        nc.scalar.activation(
                            ta[:, :],
                            py[0:CH, :],
                            AF.Tanh,
                            bias=bias_sb[0:CH, b : b + 1],
                        )
                        ts = workp.tile([CH, UW], F32R, tag="ts")
                        nc.scalar.activation(
                            ts[:, :],
                            py[CH : 2 * CH, :],
                            AF.Sigmoid,
                            bias=bias_sb[CH : 2 * CH, b : b + 1],
                        )
                        # zx = [z ; x-window]: K=128 residual matmuls do
                        # resi_w @ z + x via the [resi_w.T; I] weight tile.
                        zx = workp.tile([2 * CH, UW], F32R, tag="zx")
                        nc.vector.tensor_mul(zx[0:CH, :], ta[:, :], ts[:, :])
                        nc.sync.dma_start(
                            zx[CH : 2 * CH, :],
                            xrow[:, j * UW + PAD : j * UW + PAD + UW],
                        )
                        nc.sync.dma_start(
                            z_d[b][:, j * UW : (j + 1) * UW], zx[0:CH, :]
                        )
                        po = opsum.tile([CH, UW], F32, tag="po")
                        for h in range(UW // NT):
                            nc.tensor.matmul(
                                po[:, h * NT : (h + 1) * NT],
                                riw_sb[:, :],
                                zx[:, h * NT : (h + 1) * NT],
                                start=True,
                                stop=True,
                            )
                        ot = workp.tile([CH, UW], F32, tag="ot")
                        nc.vector.tensor_scalar_add(ot[:, :], po[:, :], rb_sb[:, 0:1])
                        nc.sync.dma_start(
                            out_d[b][:, j * UW : (j + 1) * UW], ot[:, :]
                        )

    nc.compile()
    return nc


def get_nc():
    global _NC
    if _NC is None:
        _NC = _build()
    return _NC


def make_in_maps(inputs):
    x = np.ascontiguousarray(np.asarray(inputs["x"], np.float32))
    c = np.asarray(inputs["c"], np.float32)
    aw = np.asarray(inputs["adapter_w"], np.float32)
    ab = np.asarray(inputs["adapter_b"], np.float32)
    bw = np.ascontiguousarray(np.asarray(inputs["bias_w"], np.float32))
    bb = np.asarray(inputs["bias_b"], np.float32).reshape(1, 2 * CH)
    rw = np.asarray(inputs["resi_w"], np.float32)
    rb = np.asarray(inputs["resi_b"], np.float32).reshape(CH, 1)

    # adapter columns [cond, (o,i,k)] -> [cond, (i,k,o)]
    aw_r = np.ascontiguousarray(
        aw.reshape(COND, 2 * CH, CH, K).transpose(0, 2, 3, 1).reshape(COND, F)
    )
    ab_r1 = ab.reshape(2 * CH, CH, K).transpose(1, 2, 0).reshape(CH, FI)
    ab_r = np.ascontiguousarray(np.concatenate([ab_r1, ab_r1], axis=0))
    riw = np.ascontiguousarray(
        np.concatenate([rw.T, np.eye(CH, dtype=np.float32)], axis=0)
    )

    import ml_dtypes

    adt = ml_dtypes.bfloat16 if ADAPTER_BF16 else np.float32
    aw_bf = np.ascontiguousarray(aw_r.astype(adt))
    in_maps = []
    for m in range(NCORES):
        sl = slice(BL * m, BL * (m + 1))
        in_maps.append(
            {
                "x_in": np.ascontiguousarray(x[sl]),
                "cT": np.ascontiguousarray(c[sl].T.astype(adt)),
                "cTf": np.ascontiguousarray(c[sl].T),
                "aw_r": aw_bf,
                "ab_r": ab_r,
                "bw": bw,
                "bb": bb,
                "riw": riw,
                "rb": rb,
            }
        )
    return in_maps


def kernel(**inputs):
    global LAST_RESULTS
    nc = get_nc()
    in_maps = make_in_maps(inputs)
    res = run_bass_kernel_spmd(
        nc, in_maps, list(range(NCORES)), trace=TRACE
    )
    LAST_RESULTS = res
    out = np.empty((B, CH, T), np.float32)
    z = np.empty((B, CH, T), np.float32)
    for m in range(NCORES):
        out[BL * m : BL * (m + 1)] = res.results[m]["out_d"]
        z[BL * m : BL * (m + 1)] = res.results[m]["z_d"]
    return out, z



# revision 2
# speedup vs baseline: 1.9662x; 1.9662x over previous
"""Gated TCN layer (fully conditioned) as a Bass/Tile kernel on 8 NeuronCores.

Reference computation (per sample b):
    kern = (c @ adapter_w + adapter_b).reshape(2*CH, CH, K)
    y    = dilated causal conv of x with per-sample kern (K=3, dil=4)
    y   += (c @ bias_w + bias_b)[:, None]
    z    = tanh(y[:CH]) * sigmoid(y[CH:])
    out  = resi_w @ z + resi_b + x
Returns (out, z).

Sharding: data-parallel over batch, 2 samples per core. Both samples are
stacked on the 128 SBUF partitions (rows 0-63 = sample 0, 64-127 = sample 1)
with block-diagonal per-tap dynamic kernels, so every matmul contracts over
128 partitions and every activation / vector op runs at full 128-partition
width. All heavy traffic and matmuls are bf16 (outputs are converted back to
f32 on the host); PSUM accumulation stays f32.
"""

import numpy as np

from concourse import bacc, mybir, tile
from concourse.bass_utils import run_bass_kernel_spmd

K = 3
DIL = 4
CH = 64
COND = 128
B, T = 16, 16384
NCORES = 8
BL = B // NCORES          # samples per core
PAD = (K - 1) * DIL       # causal left pad = 8
NT = 512                  # matmul free-dim (one PSUM bank of fp32)
UW = 1024                 # processing unit width (2 PSUM banks)
NJ = T // UW
F = K * CH * 2 * CH       # 24576 adapter columns
FI = 2 * CH * K           # 384 adapter columns per input-channel row
XCHUNK = 4096             # x column chunk per input DMA

F32 = mybir.dt.float32
F32R = mybir.dt.float32r
BF16 = mybir.dt.bfloat16
AF = mybir.ActivationFunctionType
ALU = mybir.AluOpType

# Set by test.py to capture a profile; harness path leaves these alone.
TRACE = False
LAST_RESULTS = None

_NC = None


def _build():
    nc = bacc.Bacc("TRN2", target_bir_lowering=False, debug=False)

    x2_d = nc.dram_tensor("x2", [2 * CH, T], BF16, kind="ExternalInput")
    cT_d = nc.dram_tensor("cT", [COND, BL], BF16, kind="ExternalInput")
    cTf_d = nc.dram_tensor("cTf", [COND, BL], F32R, kind="ExternalInput")
    aw_d = nc.dram_tensor("aw_r", [COND, F], BF16, kind="ExternalInput")
    ab_d = nc.dram_tensor("ab_r", [2 * CH, FI], F32R, kind="ExternalInput")
    bw_d = nc.dram_tensor("bw", [COND, 2 * CH], F32R, kind="ExternalInput")
    bb_d = nc.dram_tensor("bb", [1, 2 * CH], F32R, kind="ExternalInput")
    riw2_d = nc.dram_tensor("riw2", [2 * CH, 2 * CH], BF16, kind="ExternalInput")
    rb2_d = nc.dram_tensor("rb2", [2 * CH, 1], F32, kind="ExternalInput")
    z_d = nc.dram_tensor("z_d", [BL, CH, T], BF16, kind="ExternalOutput")
    out_d = nc.dram_tensor("out_d", [BL, CH, T], BF16, kind="ExternalOutput")

    with tile.TileContext(nc) as tc:
        with (
            tc.tile_pool(name="const", bufs=1) as constp,
            tc.tile_pool(name="xpool", bufs=1) as xpool,
            tc.tile_pool(name="kern", bufs=1) as kernp,
        ):
            # Both samples stacked on 128 partitions, left-padded by PAD.
            xbuf = xpool.tile([2 * CH, PAD + T], BF16)
            nc.vector.memset(xbuf[:, 0:PAD], 0.0)
            # First column chunk up front (gates conv j=0..3); the rest are
            # issued on the GpSimd queue so they don't serialize behind the
            # phase-A DMA chain on the Sync sequencer.
            nc.sync.dma_start(xbuf[:, PAD : PAD + XCHUNK], x2_d[:, 0:XCHUNK])
            for cc in range(1, T // XCHUNK):
                nc.gpsimd.dma_start(
                    xbuf[:, PAD + cc * XCHUNK : PAD + (cc + 1) * XCHUNK],
                    x2_d[:, cc * XCHUNK : (cc + 1) * XCHUNK],
                )

            cT_sb = constp.tile([COND, BL], BF16)
            nc.sync.dma_start(cT_sb[:, :], cT_d[:, :])
            cTf_sb = constp.tile([COND, BL], F32R)
            nc.sync.dma_start(cTf_sb[:, :], cTf_d[:, :])
            bw_sb = constp.tile([COND, 2 * CH], F32R)
            nc.sync.dma_start(bw_sb[:, :], bw_d[:, :])
            bb_sb = constp.tile([1, 2 * CH], F32R)
            nc.sync.dma_start(bb_sb[:, :], bb_d[:, :])
            riw2_sb = constp.tile([2 * CH, 2 * CH], BF16)
            nc.sync.dma_start(riw2_sb[:, :], riw2_d[:, :])
            rb2_sb = constp.tile([2 * CH, 1], F32)
            nc.sync.dma_start(rb2_sb[:, :], rb2_d[:, :])
            ab_sb = constp.tile([2 * CH, FI], F32R)
            nc.sync.dma_start(ab_sb[:, :], ab_d[:, :])
            ones_sb = constp.tile([1, BL], F32R)
            nc.vector.memset(ones_sb[:, :].bitcast(F32), 1.0)

            kern_raw = kernp.tile([2 * CH, FI], F32R, name="kern_raw")
            kfin = kernp.tile([2 * CH, FI], BF16, name="kfin")
            bias_sb = constp.tile([2 * CH, BL], F32)
            bias2t = constp.tile([2 * CH, 1], F32)
            bias2s = constp.tile([2 * CH, 1], F32)
            # Block-diagonal per-tap weights: rows (i of s0; i of s1), cols
            # (o-half of s0; o-half of s1). Off-diagonal quadrants stay zero.
            kdt = [kernp.tile([2 * CH, 2 * CH], BF16, name=f"kdt{k}") for k in range(K)]
            kds = [kernp.tile([2 * CH, 2 * CH], BF16, name=f"kds{k}") for k in range(K)]
            for k in range(K):
                nc.vector.memset(kdt[k][:, :], 0.0)
                nc.vector.memset(kds[k][:, :], 0.0)

            # ---------------- phase A: adapter + conditioned bias ----------
            with (
                tc.tile_pool(name="awp", bufs=3) as awp,
                tc.tile_pool(name="apsum", bufs=4, space="PSUM") as apsum,
                tc.tile_pool(name="stg", bufs=4) as stgp,
                tc.tile_pool(name="bpsum", bufs=1, space="PSUM") as bpsum,
            ):
                pb = bpsum.tile([2 * CH, BL], F32)
                nc.tensor.matmul(
                    pb[:, :], bw_sb[:, :], cTf_sb[:, :], start=True, stop=False
                )
                nc.tensor.matmul(
                    pb[:, :], bb_sb[:, :], ones_sb[:, :], start=False, stop=True
                )
                nc.vector.tensor_copy(bias_sb[:, :], pb[:, :])
                # Stacked per-half biases: bias2t[64*s + c] = bias[c, s].
                for s in range(BL):
                    nc.sync.dma_start(
                        bias2t[CH * s : CH * (s + 1), 0:1], bias_sb[0:CH, s : s + 1]
                    )
                    nc.sync.dma_start(
                        bias2s[CH * s : CH * (s + 1), 0:1],
                        bias_sb[CH : 2 * CH, s : s + 1],
                    )

                # 16 groups of 4 input-channel rows (4*FI = 1536 cols each)
                for g in range(CH // 4):
                    if g % 4 == 0:
                        awt = awp.tile([COND, 16 * FI], BF16, tag="aw")
                        nc.sync.dma_start(
                            awt[:, :], aw_d[:, g * 4 * FI : (g + 4) * 4 * FI]
                        )
                        aoff = g * 4 * FI
                    stg = stgp.tile([BL, 4 * FI], F32R, tag="stg")
                    for u in range(4):
                        j = 4 * g + u
                        ps = apsum.tile([BL, FI], F32, tag="ap")
                        nc.tensor.matmul(
                            ps[:, :],
                            cT_sb[:, :],
                            awt[:, j * FI - aoff : (j + 1) * FI - aoff],
                            start=True,
                            stop=True,
                        )
                        if u % 2 == 0:
                            nc.scalar.activation(
                                stg[:, u * FI : (u + 1) * FI], ps[:, :], AF.Copy
                            )
                        else:
                            nc.vector.tensor_copy(
                                stg[:, u * FI : (u + 1) * FI], ps[:, :]
                            )
                    for s in range(BL):
                        nc.sync.dma_start(
                            kern_raw[CH * s + 4 * g : CH * s + 4 * g + 4, :],
                            stg[s : s + 1, :],
                        )
                nc.vector.tensor_add(kfin[:, :], kern_raw[:, :], ab_sb[:, :])
                # Scatter the diagonal blocks of each tap/half weight tile.
                for k in range(K):
                    nc.sync.dma_start(
                        kdt[k][0:CH, 0:CH], kfin[0:CH, 128 * k : 128 * k + CH]
                    )
                    nc.sync.dma_start(
                        kdt[k][CH:, CH:], kfin[CH:, 128 * k : 128 * k + CH]
                    )
                    nc.sync.dma_start(
                        kds[k][0:CH, 0:CH], kfin[0:CH, 128 * k + CH : 128 * (k + 1)]
                    )
                    nc.sync.dma_start(
                        kds[k][CH:, CH:], kfin[CH:, 128 * k + CH : 128 * (k + 1)]
                    )

            # ---------------- phase B: conv + gate + residual --------------
            with (
                tc.tile_pool(name="ypsum", bufs=2, space="PSUM") as ypsum,
                tc.tile_pool(name="work", bufs=3) as workp,
            ):
                for j in range(NJ):
                    pyt = ypsum.tile([2 * CH, UW], F32, tag="pyt")
                    pys = ypsum.tile([2 * CH, UW], F32, tag="pys")
                    # k outer / h inner so each weight tile is loaded once.
                    for k in range(K):
                        for h in range(UW // NT):
                            c0 = j * UW + h * NT + DIL * k
                            nc.tensor.matmul(
                                pyt[:, h * NT : (h + 1) * NT],
                                kdt[k][:, :],
                                xbuf[:, c0 : c0 + NT],
                                start=(k == 0),
                                stop=(k == K - 1),
                            )
                    for k in range(K):
                        for h in range(UW // NT):
                            c0 = j * UW + h * NT + DIL * k
                            nc.tensor.matmul(
                                pys[:, h * NT : (h + 1) * NT],
                                kds[k][:, :],
                                xbuf[:, c0 : c0 + NT],
                                start=(k == 0),
                                stop=(k == K - 1),
                            )
                    th = workp.tile([2 * CH, UW], BF16, tag="th")
                    nc.scalar.activation(
                        th[:, :], pyt[:, :], AF.Tanh, bias=bias2t[:, 0:1]
                    )
                    sg = workp.tile([2 * CH, UW], BF16, tag="sg")
                    nc.scalar.activation(
                        sg[:, :], pys[:, :], AF.Sigmoid, bias=bias2s[:, 0:1]
                    )
                    z2 = workp.tile([2 * CH, UW], BF16, tag="z2")
                    nc.vector.tensor_mul(z2[:, :], th[:, :], sg[:, :])
                    for s in range(BL):
                        nc.sync.dma_start(
                            z_d[s][:, j * UW : (j + 1) * UW],
                            z2[CH * s : CH * (s + 1), :],
                        )
                    # Residual matmul reuses pyt's PSUM banks (WAR on tanh).
                    for h in range(UW // NT):
                        nc.tensor.matmul(
                            pyt[:, h * NT : (h + 1) * NT],
                            riw2_sb[:, :],
                            z2[:, h * NT : (h + 1) * NT],
                            start=True,
                            stop=True,
                        )
                    ot = workp.tile([2 * CH, UW], BF16, tag="ot")
                    nc.vector.scalar_tensor_tensor(
                        ot[:, :],
                        pyt[:, :],
                        rb2_sb[:, 0:1],
                        xbuf[:, PAD + j * UW : PAD + (j + 1) * UW],
                        ALU.add,
                        ALU.add,
                    )
                    for s in range(BL):
                        nc.sync.dma_start(
                            out_d[s][:, j * UW : (j + 1) * UW],
                            ot[CH * s : CH * (s + 1), :],
                        )

    nc.compile()
    return nc


def get_nc():
    global _NC
    if _NC is None:
        _NC = _build()
    return _NC


def make_in_maps(inputs):
    import ml_dtypes

    BF = ml_dtypes.bfloat16

    x = np.asarray(inputs["x"], np.float32)
    c = np.asarray(inputs["c"], np.float32)
    aw = np.asarray(inputs["adapter_w"], np.float32)
    ab = np.asarray(inputs["adapter_b"], np.float32)
    bw = np.ascontiguousarray(np.asarray(inputs["bias_w"], np.float32))
    bb = np.asarray(inputs["bias_b"], np.float32).reshape(1, 2 * CH)
    rw = np.asarray(inputs["resi_w"], np.float32)
    rb = np.asarray(inputs["resi_b"], np.float32).reshape(CH, 1)

    # adapter columns [cond, (o,i,k)] -> [cond, (i,k,o)]
    aw_r = np.ascontiguousarray(
        aw.reshape(COND, 2 * CH, CH, K).transpose(0, 2, 3, 1).reshape(COND, F)
        .astype(BF)
    )
    ab_r1 = ab.reshape(2 * CH, CH, K).transpose(1, 2, 0).reshape(CH, FI)
    ab_r = np.ascontiguousarray(np.concatenate([ab_r1, ab_r1], axis=0))
    riw2 = np.zeros((2 * CH, 2 * CH), np.float32)
    riw2[0:CH, 0:CH] = rw.T
    riw2[CH:, CH:] = rw.T
    riw2 = np.ascontiguousarray(riw2.astype(BF))
    rb2 = np.ascontiguousarray(np.concatenate([rb, rb], axis=0))
    x_bf = x.astype(BF)

    in_maps = []
    for m in range(NCORES):
        sl = slice(BL * m, BL * (m + 1))
        in_maps.append(
            {
                "x2": np.ascontiguousarray(x_bf[sl].reshape(2 * CH, T)),
                "cT": np.ascontiguousarray(c[sl].T.astype(BF)),
                "cTf": np.ascontiguousarray(c[sl].T),
                "aw_r": aw_r,
                "ab_r": ab_r,
                "bw": bw,
                "bb": bb,
                "riw2": riw2,
                "rb2": rb2,
            }
        )
    return in_maps


def kernel(**inputs):
    global LAST_RESULTS
    nc = get_nc()
    in_maps = make_in_maps(inputs)
    res = run_bass_kernel_spmd(nc, in_maps, list(range(NCORES)), trace=TRACE)
    LAST_RESULTS = res
    out = np.empty((B, CH, T), np.float32)
    z = np.empty((B, CH, T), np.float32)
    for m in range(NCORES):
        out[BL * m : BL * (m + 1)] = np.asarray(
            res.results[m]["out_d"], dtype=np.float32
        )
        z[BL * m : BL * (m + 1)] = np.asarray(res.results[m]["z_d"], dtype=np.float32)
    return out, z


# revision 7
# speedup vs baseline: 2.0910x; 1.0635x over previous
"""Gated TCN layer (fully conditioned) as a Bass/Tile kernel on 8 NeuronCores.

Reference computation (per sample b):
    kern = (c @ adapter_w + adapter_b).reshape(2*CH, CH, K)
    y    = dilated causal conv of x with per-sample kern (K=3, dil=4)
    y   += (c @ bias_w + bias_b)[:, None]
    z    = tanh(y[:CH]) * sigmoid(y[CH:])
    out  = resi_w @ z + resi_b + x
Returns (out, z).

Sharding: data-parallel over batch, 2 samples per core. Both samples are
stacked on the 128 SBUF partitions (rows 0-63 = sample 0, 64-127 = sample 1)
with block-diagonal per-tap dynamic kernels, so every matmul contracts over
128 partitions and every activation / vector op runs at full 128-partition
width. All heavy traffic and matmuls are bf16 (outputs are converted back to
f32 on the host); PSUM accumulation stays f32.
"""

import numpy as np

from concourse import bacc, mybir, tile
from concourse.bass_utils import run_bass_kernel_spmd

K = 3
DIL = 4
CH = 64
COND = 128
B, T = 16, 16384
NCORES = 8
BL = B // NCORES          # samples per core
PAD = (K - 1) * DIL       # causal left pad = 8
NT = 512                  # matmul free-dim (one PSUM bank of fp32)
UW = 1024                 # processing unit width (2 PSUM banks)
NJ = T // UW
F = K * CH * 2 * CH       # 24576 adapter columns
FI = 2 * CH * K           # 384 adapter columns per input-channel row
XCHUNK = 4096             # x column chunk per input DMA

F32 = mybir.dt.float32
F32R = mybir.dt.float32r
BF16 = mybir.dt.bfloat16
AF = mybir.ActivationFunctionType
ALU = mybir.AluOpType

# Set by test.py to capture a profile; harness path leaves these alone.
TRACE = False
LAST_RESULTS = None

_NC = None


def _build():
    nc = bacc.Bacc("TRN2", target_bir_lowering=False, debug=False)

    x2_d = nc.dram_tensor("x2", [2 * CH, T], BF16, kind="ExternalInput")
    cT_d = nc.dram_tensor("cT", [COND, BL], BF16, kind="ExternalInput")
    cTf_d = nc.dram_tensor("cTf", [COND, BL], F32R, kind="ExternalInput")
    aw_d = nc.dram_tensor("aw_r", [COND, F], BF16, kind="ExternalInput")
    ab_d = nc.dram_tensor("ab_r", [2 * CH, FI], F32R, kind="ExternalInput")
    bw_d = nc.dram_tensor("bw", [COND, 2 * CH], F32R, kind="ExternalInput")
    bb_d = nc.dram_tensor("bb", [1, 2 * CH], F32R, kind="ExternalInput")
    riw2_d = nc.dram_tensor("riw2", [2 * CH, 2 * CH], BF16, kind="ExternalInput")
    rb2_d = nc.dram_tensor("rb2", [2 * CH, 1], F32, kind="ExternalInput")
    z_d = nc.dram_tensor("z_d", [BL, CH, T], BF16, kind="ExternalOutput")
    out_d = nc.dram_tensor("out_d", [BL, CH, T], BF16, kind="ExternalOutput")

    with tile.TileContext(nc) as tc:
        with (
            tc.tile_pool(name="const", bufs=1) as constp,
            tc.tile_pool(name="xpool", bufs=1) as xpool,
            tc.tile_pool(name="kern", bufs=1) as kernp,
        ):
            # Both samples stacked on 128 partitions, left-padded by PAD.
            xbuf = xpool.tile([2 * CH, PAD + T], BF16)
            nc.vector.memset(xbuf[:, 0:PAD], 0.0)
            # First column chunk up front (gates conv j=0..3); chunks 1..3
            # are issued after phase A so the adapter weights (on the
            # critical path) get the DMA bandwidth first.
            nc.sync.dma_start(xbuf[:, PAD : PAD + XCHUNK], x2_d[:, 0:XCHUNK])

            # Warm the PE p-state during the input load: ~50 back-to-back
            # dummy matmuls keep the tensor engine continuously busy so the
            # adapter matmuls run at full clock.
            warm_w = constp.tile([2 * CH, 2 * CH], BF16)
            nc.vector.memset(warm_w[:, :], 0.0)
            warm_x = constp.tile([2 * CH, NT], BF16)
            nc.vector.memset(warm_x[:, :], 0.0)

            cT_sb = constp.tile([COND, BL], BF16)
            nc.sync.dma_start(cT_sb[:, :], cT_d[:, :])
            cTf_sb = constp.tile([COND, BL], F32R)
            nc.sync.dma_start(cTf_sb[:, :], cTf_d[:, :])
            bw_sb = constp.tile([COND, 2 * CH], F32R)
            nc.sync.dma_start(bw_sb[:, :], bw_d[:, :])
            bb_sb = constp.tile([1, 2 * CH], F32R)
            nc.sync.dma_start(bb_sb[:, :], bb_d[:, :])
            riw2_sb = constp.tile([2 * CH, 2 * CH], BF16)
            nc.sync.dma_start(riw2_sb[:, :], riw2_d[:, :])
            rb2_sb = constp.tile([2 * CH, 1], F32)
            nc.sync.dma_start(rb2_sb[:, :], rb2_d[:, :])
            ab_sb = constp.tile([2 * CH, FI], F32R)
            nc.sync.dma_start(ab_sb[:, :], ab_d[:, :])
            ones_sb = constp.tile([1, BL], F32R)
            nc.vector.memset(ones_sb[:, :].bitcast(F32), 1.0)

            kern_raw = kernp.tile([2 * CH, FI], F32R, name="kern_raw")
            bias_sb = constp.tile([2 * CH, BL], F32)
            bias2t = constp.tile([2 * CH, 1], F32)
            bias2s = constp.tile([2 * CH, 1], F32)
            # Block-diagonal per-tap weights: rows (i of s0; i of s1), cols
            # (o-half of s0; o-half of s1). Off-diagonal quadrants stay zero.
            kdt = [kernp.tile([2 * CH, 2 * CH], BF16, name=f"kdt{k}") for k in range(K)]
            kds = [kernp.tile([2 * CH, 2 * CH], BF16, name=f"kds{k}") for k in range(K)]
            for k in range(K):
                nc.vector.memset(kdt[k][:, :], 0.0)
                nc.vector.memset(kds[k][:, :], 0.0)

            # ---------------- phase A: adapter + conditioned bias ----------
            with (
                tc.tile_pool(name="awp", bufs=3) as awp,
                tc.tile_pool(name="apsum", bufs=4, space="PSUM") as apsum,
                tc.tile_pool(name="stg", bufs=4) as stgp,
                tc.tile_pool(name="bpsum", bufs=1, space="PSUM") as bpsum,
                tc.tile_pool(name="wpsum", bufs=1, space="PSUM") as wpsum,
            ):
                pw = wpsum.tile([2 * CH, NT], F32)
                for _ in range(50):
                    nc.tensor.matmul(
                        pw[:, :], warm_w[:, :], warm_x[:, :], start=True, stop=True
                    )

                pb = bpsum.tile([2 * CH, BL], F32)
                nc.tensor.matmul(
                    pb[:, :], bw_sb[:, :], cTf_sb[:, :], start=True, stop=False
                )
                nc.tensor.matmul(
                    pb[:, :], bb_sb[:, :], ones_sb[:, :], start=False, stop=True
                )
                nc.vector.tensor_copy(bias_sb[:, :], pb[:, :])
                # Stacked per-half biases: bias2t[64*s + c] = bias[c, s].
                for s in range(BL):
                    nc.sync.dma_start(
                        bias2t[CH * s : CH * (s + 1), 0:1], bias_sb[0:CH, s : s + 1]
                    )
                    nc.sync.dma_start(
                        bias2s[CH * s : CH * (s + 1), 0:1],
                        bias_sb[CH : 2 * CH, s : s + 1],
                    )

                # 16 groups of 4 input-channel rows (4*FI = 1536 cols each)
                for g in range(CH // 4):
                    if g % 4 == 0:
                        awt = awp.tile([COND, 16 * FI], BF16, tag="aw")
                        nc.sync.dma_start(
                            awt[:, :], aw_d[:, g * 4 * FI : (g + 4) * 4 * FI]
                        )
                        aoff = g * 4 * FI
                    stg = stgp.tile([BL, 4 * FI], F32R, tag="stg")
                    for u in range(4):
                        j = 4 * g + u
                        ps = apsum.tile([BL, FI], F32, tag="ap")
                        nc.tensor.matmul(
                            ps[:, :],
                            cT_sb[:, :],
                            awt[:, j * FI - aoff : (j + 1) * FI - aoff],
                            start=True,
                            stop=True,
                        )
                        if u % 2 == 0:
                            nc.scalar.activation(
                                stg[:, u * FI : (u + 1) * FI], ps[:, :], AF.Copy
                            )
                        else:
                            nc.vector.tensor_copy(
                                stg[:, u * FI : (u + 1) * FI], ps[:, :]
                            )
                    for s in range(BL):
                        nc.sync.dma_start(
                            kern_raw[CH * s + 4 * g : CH * s + 4 * g + 4, :],
                            stg[s : s + 1, :],
                        )
                # Fill the diagonal blocks of each tap/half weight tile
                # directly on the vector engine (adds adapter_b and converts
                # to bf16 in the same pass; off-diagonal quadrants stay 0).
                for k in range(K):
                    nc.vector.tensor_add(
                        kdt[k][0:CH, 0:CH],
                        kern_raw[0:CH, 128 * k : 128 * k + CH],
                        ab_sb[0:CH, 128 * k : 128 * k + CH],
                    )
                    nc.vector.tensor_add(
                        kdt[k][CH:, CH:],
                        kern_raw[CH:, 128 * k : 128 * k + CH],
                        ab_sb[CH:, 128 * k : 128 * k + CH],
                    )
                    nc.vector.tensor_add(
                        kds[k][0:CH, 0:CH],
                        kern_raw[0:CH, 128 * k + CH : 128 * (k + 1)],
                        ab_sb[0:CH, 128 * k + CH : 128 * (k + 1)],
                    )
                    nc.vector.tensor_add(
                        kds[k][CH:, CH:],
                        kern_raw[CH:, 128 * k + CH : 128 * (k + 1)],
                        ab_sb[CH:, 128 * k + CH : 128 * (k + 1)],
                    )

            # Remaining x column chunks: issued here so their descriptors
            # queue behind the adapter-weight DMAs.
            for cc in range(1, T // XCHUNK):
                nc.sync.dma_start(
                    xbuf[:, PAD + cc * XCHUNK : PAD + (cc + 1) * XCHUNK],
                    x2_d[:, cc * XCHUNK : (cc + 1) * XCHUNK],
                )

            # ---------------- phase B: conv + gate + residual --------------
            with (
                tc.tile_pool(name="ypsum", bufs=2, space="PSUM") as ypsum,
                tc.tile_pool(name="work", bufs=3) as workp,
            ):
                for j in range(NJ):
                    pyt = ypsum.tile([2 * CH, UW], F32, tag="pyt")
                    pys = ypsum.tile([2 * CH, UW], F32, tag="pys")
                    # k outer / h inner so each weight tile is loaded once.
                    for k in range(K):
                        for h in range(UW // NT):
                            c0 = j * UW + h * NT + DIL * k
                            nc.tensor.matmul(
                                pyt[:, h * NT : (h + 1) * NT],
                                kdt[k][:, :],
                                xbuf[:, c0 : c0 + NT],
                                start=(k == 0),
                                stop=(k == K - 1),
                            )
                    for k in range(K):
                        for h in range(UW // NT):
                            c0 = j * UW + h * NT + DIL * k
                            nc.tensor.matmul(
                                pys[:, h * NT : (h + 1) * NT],
                                kds[k][:, :],
                                xbuf[:, c0 : c0 + NT],
                                start=(k == 0),
                                stop=(k == K - 1),
                            )
                    th = workp.tile([2 * CH, UW], BF16, tag="th")
                    nc.scalar.activation(
                        th[:, :], pyt[:, :], AF.Tanh, bias=bias2t[:, 0:1]
                    )
                    sg = workp.tile([2 * CH, UW], BF16, tag="sg")
                    nc.scalar.activation(
                        sg[:, :], pys[:, :], AF.Sigmoid, bias=bias2s[:, 0:1]
                    )
                    z2 = workp.tile([2 * CH, UW], BF16, tag="z2")
                    nc.vector.tensor_mul(z2[:, :], th[:, :], sg[:, :])
                    for s in range(BL):
                        nc.sync.dma_start(
                            z_d[s][:, j * UW : (j + 1) * UW],
                            z2[CH * s : CH * (s + 1), :],
                        )
                    # Residual matmul reuses pyt's PSUM banks (WAR on tanh).
                    for h in range(UW // NT):
                        nc.tensor.matmul(
                            pyt[:, h * NT : (h + 1) * NT],
                            riw2_sb[:, :],
                            z2[:, h * NT : (h + 1) * NT],
                            start=True,
                            stop=True,
                        )
                    ot = workp.tile([2 * CH, UW], BF16, tag="ot")
                    nc.vector.scalar_tensor_tensor(
                        ot[:, :],
                        pyt[:, :],
                        rb2_sb[:, 0:1],
                        xbuf[:, PAD + j * UW : PAD + (j + 1) * UW],
                        ALU.add,
                        ALU.add,
                    )
                    # out stores go via the (otherwise idle) GpSimd queue so
                    # the Sync sequencer only issues the z stores.
                    for s in range(BL):
                        nc.gpsimd.dma_start(
                            out_d[s][:, j * UW : (j + 1) * UW],
                            ot[CH * s : CH * (s + 1), :],
                        )

    nc.compile()
    return nc


def get_nc():
    global _NC
    if _NC is None:
        _NC = _build()
    return _NC


def make_in_maps(inputs):
    import ml_dtypes

    BF = ml_dtypes.bfloat16

    x = np.asarray(inputs["x"], np.float32)
    c = np.asarray(inputs["c"], np.float32)
    aw = np.asarray(inputs["adapter_w"], np.float32)
    ab = np.asarray(inputs["adapter_b"], np.float32)
    bw = np.ascontiguousarray(np.asarray(inputs["bias_w"], np.float32))
    bb = np.asarray(inputs["bias_b"], np.float32).reshape(1, 2 * CH)
    rw = np.asarray(inputs["resi_w"], np.float32)
    rb = np.asarray(inputs["resi_b"], np.float32).reshape(CH, 1)

    # adapter columns [cond, (o,i,k)] -> [cond, (i,k,o)]
    aw_r = np.ascontiguousarray(
        aw.reshape(COND, 2 * CH, CH, K).transpose(0, 2, 3, 1).reshape(COND, F)
        .astype(BF)
    )
    ab_r1 = ab.reshape(2 * CH, CH, K).transpose(1, 2, 0).reshape(CH, FI)
    ab_r = np.ascontiguousarray(np.concatenate([ab_r1, ab_r1], axis=0))
    riw2 = np.zeros((2 * CH, 2 * CH), np.float32)
    riw2[0:CH, 0:CH] = rw.T
    riw2[CH:, CH:] = rw.T
    riw2 = np.ascontiguousarray(riw2.astype(BF))
    rb2 = np.ascontiguousarray(np.concatenate([rb, rb], axis=0))
    x_bf = x.astype(BF)

    in_maps = []
    for m in range(NCORES):
        sl = slice(BL * m, BL * (m + 1))
        in_maps.append(
            {
                "x2": np.ascontiguousarray(x_bf[sl].reshape(2 * CH, T)),
                "cT": np.ascontiguousarray(c[sl].T.astype(BF)),
                "cTf": np.ascontiguousarray(c[sl].T),
                "aw_r": aw_r,
                "ab_r": ab_r,
                "bw": bw,
                "bb": bb,
                "riw2": riw2,
                "rb2": rb2,
            }
        )
    return in_maps


def kernel(**inputs):
    global LAST_RESULTS
    nc = get_nc()
    in_maps = make_in_maps(inputs)
    res = run_bass_kernel_spmd(nc, in_maps, list(range(NCORES)), trace=TRACE)
    LAST_RESULTS = res
    out = np.empty((B, CH, T), np.float32)
    z = np.empty((B, CH, T), np.float32)
    for m in range(NCORES):
        out[BL * m : BL * (m + 1)] = np.asarray(
            res.results[m]["out_d"], dtype=np.float32
        )
        z[BL * m : BL * (m + 1)] = np.asarray(res.results[m]["z_d"], dtype=np.float32)
    return out, z
